# revision 67
# baseline (speedup 1.0000x reference)
"""Trainium2 Bass kernel for nn_CausalSelfAttention_35931696398729.

Sharding: 8 cores = (batch b in {0,1}) x (kv-head n in {0..3}).
Each core computes its 4 query heads' causal GQA attention for its batch
plus the partial c_proj (rows of Wo for its heads); the host sums the 4
partials per batch.  No device collectives.

All matmul operands are fp16 (1 cyc/row in the cost model, like bf16,
with 8x lower quantization error).  PSUM stays f32.

Key structure:
 - qT/kT (d on partitions, t free) so scores come out as ST (keys on
   partitions, queries free) and PV consumes exp(ST) directly.
 - V is projected directly in [t, d] layout (x block as the stationary
   operand) -- no PE transposes.
 - QK RMSNorm: squares are taken from the PRE-RoPE psum (rotation
   preserves column norms).  q-side factor rq(t)/sqrt(HD) is multiplied
   into q during phase 1 (Pool partition_broadcast + DVE mul); k-side
   factor rk(s) rides the Exp activation's per-partition scale;
   gamma_q*gamma_k is folded into the K RoPE tables on the host.
 - softmax runs without max-subtraction but with a constant -2 bias in
   the exponent (softmax-invariant) so exp stays in fp16 range.
 - softmax denominator: P_acc += p on DVE (fp16 tensor_add, 2x mode),
   then one [1,TC] matmul per (head, q-chunk) into a buffer borrowed
   from the score psum pool; 1/rowsum is applied to OT columns.
 - attention runs on head pairs with a 4-deep score-psum pipeline; exp
   runs per head with the per-key rk as the activation scale.
 - c_proj for q-chunk qc-1 is interleaved into attention of qc at key-
   block granularity (filler between scores and PVs hides exp latency);
   the final chunk's c_proj borrows the idle score/ot psum pools.
 - weights are host-prearranged to the SBUF layout so every DMA moves
   contiguous >=4KB rows (sub-512B segments pay 2x in the DMA engine);
   RoPE tables load behind the second x chunk; the Exp act-table load is
   prefetched before the last chunk's V matmuls.
 - y is written fp16; the host upcasts and sums the four partials.
"""

import os
import sys

sys.path.insert(0, "/opt/trn_rl_repo")

import numpy as np

import concourse.bacc as bacc
import concourse.mybir as mybir
import concourse.tile as tile
from concourse import bass_utils

B, T, D = 2, 2048, 2048
NH, NKV, HD = 16, 4, 128
G = NH // NKV  # query heads per core
EPS = 1e-6
THETA = 10000.0
N_CORES = 8
P = 128
TC = 512            # q-chunk for attention / c_proj column chunk
NTC = T // TC       # 4
TC1 = 256           # t-chunk for phase-1 projections
NTC1 = T // TC1     # 8
NKT = D // P        # 16 contraction chunks
NTB = T // P        # 16 t-blocks
EXP_BIAS = -2.0     # constant exponent shift (softmax invariant)

F32 = mybir.dt.float32
DT = mybir.dt.float16
NP_DT = np.float16


def build_program():
    nc = bacc.Bacc("TRN2", target_bir_lowering=False, debug=False,
                   enable_asserts=False, num_devices=N_CORES)

    xT = nc.dram_tensor("xT", (D, T), DT, kind="ExternalInput").ap()
    # weights host-prearranged to SBUF layout (contiguous >=4KB DMA rows)
    wq = nc.dram_tensor("wq", (P, G, NKT, HD), DT, kind="ExternalInput").ap()
    wk = nc.dram_tensor("wk", (P, NKT, HD), DT, kind="ExternalInput").ap()
    wv = nc.dram_tensor("wv", (P, NKT, HD), DT, kind="ExternalInput").ap()
    wo = nc.dram_tensor("wo", (G * HD, D), DT, kind="ExternalInput").ap()
    cosq = nc.dram_tensor("cosq", (P, T), DT, kind="ExternalInput").ap()
    sinq = nc.dram_tensor("sinq", (P, T), DT, kind="ExternalInput").ap()
    cosk = nc.dram_tensor("cosk", (P, T), DT, kind="ExternalInput").ap()
    sink = nc.dram_tensor("sink", (P, T), DT, kind="ExternalInput").ap()
    y = nc.dram_tensor("y", (T, D), DT, kind="ExternalOutput").ap()

    AF = mybir.ActivationFunctionType
    ALU = mybir.AluOpType

    with tile.TileContext(nc) as tc, \
         nc.allow_low_precision(reason="fp16 matmul/softmax pipeline"):
        with tc.tile_pool(name="persist", bufs=1) as persist, \
             tc.tile_pool(name="stri", bufs=4) as stri:
            cosq_sb = persist.tile([P, T], DT)
            sinq_sb = persist.tile([P, T], DT)
            cosk_sb = persist.tile([P, T], DT)
            sink_sb = persist.tile([P, T], DT)
            ones_col = persist.tile([P, 1], DT)
            nc.vector.memset(ones_col, 1.0)
            warm_src = persist.tile([P, P], DT)
            nc.vector.memset(warm_src, 0.0)
            eps_k = persist.tile([P, 1], F32)
            nc.vector.memset(eps_k, EPS)
            eps_q = persist.tile([1, 1], F32)
            nc.vector.memset(eps_q, HD * EPS)
            eps_q128 = persist.tile([P, 1], F32)
            nc.vector.memset(eps_q128, HD * EPS)
            expb = persist.tile([P, 1], F32)
            nc.vector.memset(expb, EXP_BIAS)
            dume = persist.tile([1, 1], F32)

            TH = T // 2
            q_sb = [[persist.tile([P, TH], DT, tag=f"q_sb{h}{a}",
                                  name=f"q_sb{h}{a}") for a in range(2)]
                    for h in range(G)]
            kT_sb = [persist.tile([P, TH], DT, tag=f"kT{a}", name=f"kT{a}")
                     for a in range(2)]
            v_sb = [persist.tile([P, NTB // 2, P], DT, tag=f"v{a}",
                                 name=f"v{a}") for a in range(2)]
            rk_tiles = [persist.tile([P, NTB // 2], F32, tag=f"rk{a}",
                                     name=f"rk{a}") for a in range(2)]
            wo_sb = persist.tile([P, G, D], DT)
            otn_sb = [persist.tile([P, T], DT, tag=f"otn{h}", name=f"otn{h}")
                      for h in range(G)]

            # PE warm-up: keep PE busy through the cold-clock ramp window
            # while the first x chunks stream in.
            with tc.tile_pool(name="warm", bufs=1, space="PSUM") as ps_w:
                warm_ps = ps_w.tile([1, P], F32)
                for _ in range(40):
                    nc.tensor.matmul(warm_ps, ones_col, warm_src,
                                     start=True, stop=True)

            # ---------------- Phase 1: projections + RoPE + norms -----------
            with tc.tile_pool(name="weights", bufs=1) as wpool, \
                 tc.tile_pool(name="xts", bufs=4) as xpool, \
                 tc.tile_pool(name="p1tmp", bufs=4) as tmpool, \
                 tc.tile_pool(name="p1q", bufs=8) as qpool, \
                 tc.tile_pool(name="p1ps", bufs=3, space="PSUM") as ps_a, \
                 tc.tile_pool(name="p1psv", bufs=2, space="PSUM") as ps_v, \
                 tc.tile_pool(name="p1sc", bufs=1, space="PSUM") as ps_sc, \
                 tc.tile_pool(name="p1sq", bufs=2, space="PSUM") as ps_sq:
                wq_sb = wpool.tile([P, G, NKT, HD], DT)
                wk_sb = wpool.tile([P, NKT, HD], DT)
                wv_sb = wpool.tile([P, NKT, HD], DT)
                nc.sync.dma_start(out=wk_sb, in_=wk)

                xts_tiles = {}

                def get_xts(i):
                    if i in xts_tiles or i >= NTC1:
                        return
                    xts = xpool.tile([P, NKT, TC1], DT, tag="xts",
                                     name="xts")
                    sl_i = slice(i * TC1, (i + 1) * TC1)
                    xre = xT[:, sl_i].rearrange("(kt p) m -> p kt m", p=P)
                    for kg in range(4):
                        nc.sync.dma_start(
                            out=xts[:, 4 * kg:4 * (kg + 1), :],
                            in_=xre[:, 4 * kg:4 * (kg + 1), :])
                    xts_tiles[i] = xts

                # startup staging on the SP DMA queue: PE-critical first,
                # tables behind the second x chunk (only DVE ropes wait)
                get_xts(0)
                for h in range(G):
                    nc.sync.dma_start(out=wq_sb[:, h], in_=wq[:, h])
                nc.sync.dma_start(out=wv_sb, in_=wv)
                get_xts(1)
                nc.sync.dma_start(out=cosk_sb, in_=cosk)
                nc.sync.dma_start(out=sink_sb, in_=sink)
                get_xts(2)
                nc.sync.dma_start(out=cosq_sb, in_=cosq)
                nc.sync.dma_start(out=sinq_sb, in_=sinq)

                def swap_copy(psb, tag):
                    # halves-swapped copy (single-input ops may cross
                    # partition bases; two-input SB+SB ops may not)
                    psb_sw = tmpool.tile([P, TC1], DT, tag=tag, name=tag)
                    nc.vector.tensor_copy(out=psb_sw[0:64, :],
                                          in_=psb[64:128, :])
                    nc.vector.tensor_copy(out=psb_sw[64:128, :],
                                          in_=psb[0:64, :])
                    return psb_sw

                def rope(psb, psb_sw, cos_t, sin_t, dst):
                    # dst = psb * cos + swap(psb) * sin   (all fp16 SBUF,
                    # partition-aligned; sin table carries the sign fold)
                    tmp = tmpool.tile([P, TC1], DT, tag="ropetmp",
                                      name="ropetmp")
                    nc.vector.tensor_mul(out=tmp, in0=psb_sw, in1=sin_t)
                    tmp2 = tmpool.tile([P, TC1], DT, tag="ropetmp2",
                                       name="ropetmp2")
                    nc.vector.tensor_mul(out=tmp2, in0=psb, in1=cos_t)
                    nc.vector.tensor_add(out=dst, in0=tmp2, in1=tmp)

                for tc_i in range(NTC1):
                    half = tc_i // (NTC1 // 2)
                    sl = slice((tc_i % (NTC1 // 2)) * TC1,
                               (tc_i % (NTC1 // 2) + 1) * TC1)
                    get_xts(tc_i + 1)
                    if tc_i == 3:
                        nc.sync.dma_start(
                            out=wo_sb,
                            in_=wo.rearrange("(h p) m -> p h m", p=P))
                    xts = xts_tiles.pop(tc_i)

                    # ---- PE: projections (K, Q heads, V) -------------------
                    ps_k = ps_a.tile([P, TC1], F32, tag="proj", name="ps_k")
                    for kt in range(NKT):
                        nc.tensor.matmul(ps_k, wk_sb[:, kt, :],
                                         xts[:, kt, :],
                                         start=(kt == 0), stop=(kt == NKT - 1))
                    # Act: psum -> fp16 SBUF copy + square (pre-RoPE norm)
                    psb_k = tmpool.tile([P, TC1], DT, tag="psb", name="psb_k")
                    nc.scalar.copy(out=psb_k, in_=ps_k)
                    psw_k = swap_copy(psb_k, "psw")
                    sq_k = tmpool.tile([P, TC1], DT, tag="sq", name="sq_k")
                    nc.scalar.square(out=sq_k, in_=psb_k)
                    gsl = slice(tc_i * TC1, (tc_i + 1) * TC1)
                    rope(psb_k, psw_k, cosk_sb[:, gsl], sink_sb[:, gsl],
                         kT_sb[half][:, sl])

                    last = tc_i == NTC1 - 1
                    q_ps = []
                    deferred = []
                    for h in range(G):
                        ps_q = ps_a.tile([P, TC1], F32, tag="proj",
                                         name="ps_q")
                        for kt in range(NKT):
                            nc.tensor.matmul(
                                ps_q, wq_sb[:, h, kt, :],
                                xts[:, kt, :],
                                start=(kt == 0), stop=(kt == NKT - 1))
                        sq_q = qpool.tile([P, TC1], DT, tag="sqq",
                                          name="sq_q")
                        qr = qpool.tile([P, TC1], DT, tag="ropeq", name="qr")
                        if last and h >= 1:
                            # defer the copy/rope chain past the norm sqrts
                            # so the Act queue frees the norm psum banks
                            # (phase 2's first scores wait on them) early;
                            # square straight from psum
                            nc.scalar.square(out=sq_q, in_=ps_q)

                            def _fin(ps_q=ps_q, sq_q=sq_q, qr=qr, h=h):
                                psb_q = qpool.tile([P, TC1], DT, tag="psbq",
                                                   name="psb_q")
                                nc.scalar.copy(out=psb_q, in_=ps_q)
                                psw_q = swap_copy(psb_q, "pswq")
                                rope(psb_q, psw_q, cosq_sb[:, gsl],
                                     sinq_sb[:, gsl], qr)
                            deferred.append(_fin)
                        else:
                            psb_q = qpool.tile([P, TC1], DT, tag="psbq",
                                               name="psb_q")
                            nc.scalar.copy(out=psb_q, in_=ps_q)
                            psw_q = swap_copy(psb_q, "pswq")
                            nc.scalar.square(out=sq_q, in_=psb_q)
                            rope(psb_q, psw_q, cosq_sb[:, gsl],
                                 sinq_sb[:, gsl], qr)
                        q_ps.append((sq_q, qr))

                    def emit_v(tc_i=tc_i, half=half, xts=xts):
                        # V directly in [t, d] layout: x block stationary
                        for i in range(TC1 // P):
                            ps_vt = ps_v.tile([P, P], F32, tag="vt",
                                              name="ps_vt")
                            for kt in range(NKT):
                                nc.tensor.matmul(
                                    ps_vt, xts[:, kt, i * P:(i + 1) * P],
                                    wv_sb[:, kt, :],
                                    start=(kt == 0), stop=(kt == NKT - 1))
                            nc.scalar.copy(
                                out=v_sb[half][:, (tc_i % (NTC1 // 2)) *
                                               (TC1 // P) + i, :],
                                in_=ps_vt)

                    if not last:
                        emit_v()

                    # ---- norm reductions (PE, end of chunk) ----------------
                    # K: per key-block column sums of sq_k (sq stationary)
                    kb0 = (tc_i % (NTC1 // 2)) * (TC1 // P)
                    # one psum bank per chunk for all norm reductions:
                    # ssq rows at partition bases 0/32/64/96 (cols 0:TC1),
                    # ssqc columns at TC1:TC1+2
                    ssqc = ps_sc.tile([P, TC1 // P], F32, tag="ssqc",
                                      name="ssqc")
                    for i in range(TC1 // P):
                        nc.tensor.matmul(ssqc[:, i:i + 1],
                                         sq_k[:, i * P:(i + 1) * P],
                                         ones_col, start=True, stop=True)
                    nc.scalar.activation(
                        out=rk_tiles[half][:, kb0:kb0 + TC1 // P], in_=ssqc,
                        func=AF.Sqrt, bias=eps_k[:], scale=float(1.0 / HD))
                    nc.vector.reciprocal(
                        out=rk_tiles[half][:, kb0:kb0 + TC1 // P],
                        in_=rk_tiles[half][:, kb0:kb0 + TC1 // P])
                    # Q: rq = 1/sqrt(ssq + HD*eps) applied to q columns
                    for h in range(G):
                        sq_q, qr = q_ps[h]
                        ssq = ps_sq.tile([1, TC1], F32, tag="ssq",
                                         name="ssq")
                        nc.tensor.matmul(ssq, ones_col, sq_q,
                                         start=True, stop=True)
                        sq_s = stri.tile([1, TC1], F32, tag="sqs",
                                         name="sq_s")
                        nc.scalar.activation(out=sq_s, in_=ssq, func=AF.Sqrt,
                                             bias=eps_q[:], scale=1.0)
                        rq = stri.tile([1, TC1], DT, tag="rq", name="rq")
                        nc.vector.reciprocal(out=rq, in_=sq_s)
                        rqB = tmpool.tile([P, TC1], DT, tag="rqB", name="rqB")
                        nc.gpsimd.partition_broadcast(rqB, rq)
                        if last and h >= 1:
                            # temper rides the deferred rope chain
                            def _tmp(h=h, qr=qr, rqB=rqB, half=half, sl=sl):
                                nc.vector.tensor_mul(
                                    out=q_sb[h][half][:, sl],
                                    in0=qr, in1=rqB)
                            deferred.append(_tmp)
                        else:
                            nc.vector.tensor_mul(out=q_sb[h][half][:, sl],
                                                 in0=qr, in1=rqB)
                    if last:
                        # prefetch the Exp act-table right after the final
                        # sqrts (Sqrt and Exp live in different table sets),
                        # then the deferred copy/rope chains and V
                        nc.scalar.activation(out=dume, in_=eps_q,
                                             func=AF.Exp, bias=expb[0:1],
                                             scale=0.0)
                        for fin in deferred:
                            fin()
                        emit_v()

            # ---------------- Phase 2: attention + c_proj -------------------
            with tc.tile_pool(name="attn", bufs=4) as apool, \
                 tc.tile_pool(name="pb", bufs=8) as ppool, \
                 tc.tile_pool(name="pacc", bufs=4) as accpool, \
                 tc.tile_pool(name="ysb", bufs=6) as ypool, \
                 tc.tile_pool(name="p2st", bufs=4, space="PSUM") as ps_st, \
                 tc.tile_pool(name="p2ot", bufs=2, space="PSUM") as ps_ot, \
                 tc.tile_pool(name="p3ya", bufs=1, space="PSUM") as ps_ya, \
                 tc.tile_pool(name="p3yb", bufs=1, space="PSUM") as ps_yb:

                def cproj_steps(qc, y_on_act=False, borrow=False):
                    # 8 emission closures for q-chunk qc's 4 t-blocks
                    steps = []
                    for tb in range(4 * qc, 4 * qc + 4):
                        for jg in (0, 2):
                            def step(tb=tb, jg=jg):
                                if borrow:
                                    # attention is done: use the idle st/ot
                                    # pools for a deeper psum ping-pong
                                    ya = ps_st.tile([P, TC], F32, tag="st",
                                                    name="ya_b")
                                    yb = ps_ot.tile([P, TC], F32, tag="ot",
                                                    name="yb_b")
                                else:
                                    ya = ps_ya.tile([P, TC], F32, tag="ya",
                                                    name="ya")
                                    yb = ps_yb.tile([P, TC], F32, tag="yb",
                                                    name="yb")
                                for h in range(G):
                                    lhs = otn_sb[h][:, tb * P:(tb + 1) * P]
                                    nc.tensor.matmul(
                                        ya, lhs,
                                        wo_sb[:, h, jg * TC:(jg + 1) * TC],
                                        start=(h == 0), stop=(h == G - 1))
                                    nc.tensor.matmul(
                                        yb, lhs,
                                        wo_sb[:, h,
                                              (jg + 1) * TC:(jg + 2) * TC],
                                        start=(h == 0), stop=(h == G - 1))
                                for j, yp in ((jg, ya), (jg + 1, yb)):
                                    y_sb = ypool.tile([P, TC], DT, tag="y_sb",
                                                      name="y_sb")
                                    if y_on_act:
                                        nc.scalar.copy(out=y_sb, in_=yp)
                                    else:
                                        nc.vector.tensor_copy(out=y_sb,
                                                              in_=yp)
                                    nc.sync.dma_start(
                                        out=y[tb * P:(tb + 1) * P,
                                              j * TC:(j + 1) * TC],
                                        in_=y_sb)
                            steps.append(step)
                    return steps

                for qc in range(NTC):
                    qsl = slice(qc * TC, (qc + 1) * TC)
                    nkb = 4 * (qc + 1)
                    steps = cproj_steps(qc - 1) if qc > 0 else []
                    # interleave points: 2 mid-loop per pair + pair ends
                    mids = {max(1, nkb // 3), max(2, (2 * nkb) // 3)}
                    for pair in ((0, 1), (2, 3)):
                        ot_ps = {}
                        acc = {}
                        for h in pair:
                            ot_ps[h] = ps_ot.tile([P, TC], F32, tag="ot",
                                                  name="ot_ps")
                            acc[h] = accpool.tile([P, TC], DT, tag="acc",
                                                  name="acc")
                        def score_mms(kb):
                            c0_ = max(kb - 4 * qc, 0) * P
                            kbh, kbl = divmod(kb, NTB // 2)
                            qh = qc // 2
                            qof = (qc % 2) * TC
                            d = {}
                            for h in pair:
                                st = ps_st.tile([P, TC], F32, tag="st",
                                                name="st_ps")
                                nc.tensor.matmul(
                                    st[:, c0_:],
                                    kT_sb[kbh][:, kbl * P:(kbl + 1) * P],
                                    q_sb[h][qh][:, qof + c0_:qof + TC],
                                    start=True, stop=True)
                                d[h] = st
                            return d

                        # first processed pair: run scores one kb ahead so
                        # PE works through the Exp act-table load latency
                        lead = qc == 0 and pair == (0, 1)
                        if lead:
                            pre = score_mms(0)
                            # keep the PE clock hot through the Exp-table
                            # load wait; the real PV's start=True zeroes
                            # this region afterwards
                            for _ in range(24):
                                nc.tensor.matmul(
                                    ot_ps[pair[0]][0:1, 0:P], ones_col,
                                    warm_src, start=True, stop=True)
                        for kb in range(nkb):
                            r = kb - 4 * qc  # >=0 on diagonal blocks
                            c0 = max(r, 0) * P  # first valid q column
                            if lead:
                                sts = pre
                                if kb + 1 < nkb:
                                    pre = score_mms(kb + 1)
                            else:
                                sts = score_mms(kb)
                            ps_ = {}
                            for h in pair:
                                p_ = ppool.tile([P, TC], DT, tag="p",
                                                name="p_sb")
                                nc.scalar.activation(
                                    out=p_[:, c0:], in_=sts[h][:, c0:],
                                    func=AF.Exp, bias=expb[:],
                                    scale=rk_tiles[kb // (NTB // 2)][
                                        :, kb % (NTB // 2):
                                        kb % (NTB // 2) + 1])
                                if r >= 0:
                                    # causal mask on the diagonal strip
                                    nc.gpsimd.affine_select(
                                        out=p_[:, c0:c0 + P],
                                        in_=p_[:, c0:c0 + P],
                                        pattern=[[1, P]],
                                        compare_op=ALU.is_ge,
                                        fill=0.0,
                                        base=0,
                                        channel_multiplier=-1)
                                if kb == 0:
                                    nc.vector.tensor_copy(out=acc[h],
                                                          in_=p_)
                                else:
                                    nc.vector.tensor_add(
                                        out=acc[h][:, c0:],
                                        in0=acc[h][:, c0:],
                                        in1=p_[:, c0:])
                                ps_[h] = p_
                            # cproj filler between scores and PVs hides
                            # the exp latency on the PE queue
                            if steps and kb in mids:
                                steps.pop(0)()
                            for h in pair:
                                nc.tensor.matmul(
                                    ot_ps[h][:, c0:],
                                    v_sb[kb // (NTB // 2)][
                                        :, kb % (NTB // 2), :],
                                    ps_[h][:, c0:], start=(kb == 0),
                                    stop=(kb == nkb - 1))
                        for h in pair:
                            # rs tile borrows a st-pool buffer
                            rs = ps_st.tile([P, TC], F32, tag="st",
                                            name="rs_ps")
                            nc.tensor.matmul(rs[0:1, :], ones_col, acc[h],
                                             start=True, stop=True)
                            recip = stri.tile([1, TC], DT, tag="recip",
                                              name="recip")
                            nc.vector.reciprocal(out=recip, in_=rs[0:1, :])
                            recipB = apool.tile([P, TC], DT, tag="recipB",
                                                name="recipB")
                            nc.gpsimd.partition_broadcast(recipB, recip)
                            # otn ahead of injected y copies in DVE queue
                            nc.vector.tensor_mul(out=otn_sb[h][:, qsl],
                                                 in0=ot_ps[h], in1=recipB)
                            if steps and not (qc == NTC - 1
                                              and pair == (2, 3)):
                                steps.pop(0)()
                    while steps:
                        steps.pop(0)()
                    prev_qc = qc
                # last-processed chunk's c_proj
                for step in cproj_steps(prev_qc, borrow=True):
                    step()

    nc.compile()
    return nc


_NC_CACHE = None


def _get_program():
    global _NC_CACHE
    if _NC_CACHE is None:
        _NC_CACHE = build_program()
    return _NC_CACHE


def _make_tables(pos, gamma2):
    half = HD // 2
    inv_freq = 1.0 / (THETA ** (np.arange(half, dtype=np.float64) / half))
    ang = (pos + np.arange(T, dtype=np.float64))[None, :] * inv_freq[:, None]
    cos = np.cos(ang)
    sin = np.sin(ang)
    cosq = np.concatenate([cos, cos], axis=0)
    sinq = np.concatenate([-sin, sin], axis=0)
    g2 = gamma2.astype(np.float64).reshape(P, 1)
    return (np.ascontiguousarray(cosq.astype(NP_DT)),
            np.ascontiguousarray(sinq.astype(NP_DT)),
            np.ascontiguousarray((cosq * g2).astype(NP_DT)),
            np.ascontiguousarray((sinq * g2).astype(NP_DT)))


def kernel(x, Wq, Wk, Wv, Wo, q_gamma, k_gamma, pos):
    x = np.asarray(x, dtype=np.float32)
    Wq = np.asarray(Wq, dtype=np.float32)
    Wk = np.asarray(Wk, dtype=np.float32)
    Wv = np.asarray(Wv, dtype=np.float32)
    Wo = np.asarray(Wo, dtype=np.float32)
    q_gamma = np.asarray(q_gamma, dtype=np.float32)
    k_gamma = np.asarray(k_gamma, dtype=np.float32)
    pos = int(np.asarray(pos))

    gamma2 = q_gamma * k_gamma
    cosq, sinq, cosk, sink = _make_tables(pos, gamma2)

    def st(a):
        return np.ascontiguousarray(a.astype(NP_DT))

    def wqr(a):
        # (D, G*HD) -> (P, G, NKT, HD): [p, h, kt, :] = a[kt*P + p, h*HD:]
        return st(a.reshape(NKT, P, G, HD).transpose(1, 2, 0, 3))

    def wkvr(a):
        # (D, HD) -> (P, NKT, HD)
        return st(a.reshape(NKT, P, HD).transpose(1, 0, 2))

    xTs = [st(x[b].T) for b in range(B)]
    in_maps = []
    for c in range(N_CORES):
        b, n = divmod(c, NKV)
        in_maps.append({
            "xT": xTs[b],
            "wq": wqr(Wq[:, n * G * HD:(n + 1) * G * HD]),
            "wk": wkvr(Wk[:, n * HD:(n + 1) * HD]),
            "wv": wkvr(Wv[:, n * HD:(n + 1) * HD]),
            "wo": st(Wo[n * G * HD:(n + 1) * G * HD, :]),
            "cosq": cosq,
            "sinq": sinq,
            "cosk": cosk,
            "sink": sink,
        })

    nc = _get_program()
    res = bass_utils.run_bass_kernel_spmd(nc, in_maps,
                                          core_ids=list(range(N_CORES)))
    out = np.zeros((B, T, D), dtype=np.float32)
    for c in range(N_CORES):
        b = c // NKV
        out[b] += res.results[c]["y"].astype(np.float32)
    return out


if __name__ == "__main__":
    build_program()
    print("program built OK")
